# revision 14
# baseline (speedup 1.0000x reference)
"""Trainium2 Bass kernel for DifferentiableRankIntegration (grid-factorized).

Math (per query row i, B=1024):
  sig[k,j] = sigmoid((s[i,k] - s[i,j]) / tau),  tau = 0.1
  Sp[j] = sum_k pos[i,k]*sig[k,j],  Sn[j] = sum_k neg[i,k]*sig[k,j]
  rank[j] = 1 + Sn[j] if pos[i,j] else 1 + Sp[j]
  out[i,j] = (K+1) * (w_v/(K+rank_v) + w_l/(K+rank_l)),  K = 60

Direct evaluation is 2*B^2 sigmoids per row on the ACT engine - the
baseline ran AT the ACT roofline (~1.76 ms). This kernel factorizes the
soft-count function f(x) = sum_k m_k sigmoid(10(s_k - x)) through a
G=128-point grid:
  pass 1: F[m] = sum_k m_k sigmoid(10(s_k - g_m))    (ACT [128,G] x8 + PE)
  filter: F' = Td F, Td = (5-tap sharpening) @ (first difference), a
          constant banded matrix applied with one tiny PE matmul (this
          cancels the smoothing bias of the reconstruction kernel; taps
          least-squares fit offline, rel err of the full output ~4e-5)
  pass 2: f(s_j) ~= sum_m F'_m sigmoid((c_m - s_j)/w), w = 0.7h, via one
          ACT instr [128, 2048] (both matrices) + PE matmuls into PSUM.
ACT work per row drops 16*1024-free -> 16*128-free + 1*2048-free (~4x).

Per-core layout (128 rows per core, 8 cores), per row r:
  stage DMA + GPSIMD partition_broadcast bc2[128, 2048] = [s_v row|s_l row]
  pass-1: ACT instrs have a large fixed overhead on silicon, so the
          (k, m) outer difference is built by ONE DVE op with broadcast
          access patterns - diff[p,(q,c,m)] = sT10c[p, q*1024+c*128+r]
          (stride-0 inner repeat) - 10*g_m (stride-0 chunk repeat) -
          and sigmoided by ONE ACT instr [128, 2048] covering both
          matrices; 16 tiny PE matmuls (N=2) contract with wint pairs.
  Fsb f32 copy <- Fps; Fps2 = tdT.T @ Fsb (f32 matmul); F2sb bf16 <- Fps2
  pass-2: phi = Sigmoid(-bc2/w + c_m/w) [128, 2048] bf16 (one ACT instr)
          acc2[2, 2048] = F2sb[:, q:q+2].T @ phi slices (4 matmuls)
  evict: DVE copy stg <- acc2 (GPSIMD cannot touch PSUM), then 4
  SBUF->SBUF row-scatter DMAs.
Finals are batched [128, 1024] VectorE ops + reciprocal, as before.
"""

import numpy as np

B = 1024
NCORES = 8
ROWS = B // NCORES  # 128 rows per core
P = 128
NCHUNK = B // P  # 8
TAU = 0.1
K = 60.0
SCALE = 1.0 / TAU  # 10

G = 128
LO, HI = -6.5, 6.5
H = (HI - LO) / (G - 1)
W = 0.7 * H
TAPS = (5.0404, -2.4113, 0.3911)


def _build_consts():
    g = (LO + H * np.arange(G)).astype(np.float64)
    grid10 = np.tile((10.0 * g)[None, :], (P, 1)).astype(np.float32)
    # first-difference matrix D: (DF)_m = F_m - F_{m+1}, m = 0..G-2
    D = np.zeros((G - 1, G), np.float64)
    D[np.arange(G - 1), np.arange(G - 1)] = 1.0
    D[np.arange(G - 1), np.arange(1, G)] = -1.0
    # 5-tap sharpening with edge-clipped indices
    c0, c1, c2 = TAPS
    Tb = np.zeros((G - 1, G - 1), np.float64)
    idx = np.arange(G - 1)
    for d, c in ((0, c0), (-1, c1), (1, c1), (-2, c2), (2, c2)):
        np.add.at(Tb, (idx, np.clip(idx + d, 0, G - 2)), c)
    Tfull = Tb @ D  # [G-1, G]
    tdT = np.zeros((G, G), np.float64)
    tdT[:, : G - 1] = Tfull.T
    # pass-2 per-partition bias: c_m / w (kernel centers at grid midpoints);
    # partition G-1 is a dead kernel (bias -300 -> sigmoid == 0)
    bias2 = np.full((G, 1), -300.0, np.float64)
    bias2[: G - 1, 0] = (g[:-1] + H / 2) / W
    return grid10, tdT.astype(np.float32), bias2.astype(np.float32)


GRID10, TDT, BIAS2 = _build_consts()


def _build_bass():
    import concourse.bacc as bacc
    import concourse.mybir as mybir
    from concourse.tile import TileContext

    f32 = mybir.dt.float32
    bf16 = mybir.dt.bfloat16

    nc = bacc.Bacc()

    # Per-core inputs (host pre-sharded / pre-transposed):
    sv = nc.declare_dram_parameter("sv", [ROWS, B], f32, isOutput=False)
    sl = nc.declare_dram_parameter("sl", [ROWS, B], f32, isOutput=False)
    # sT10c[p, q*1024 + c*128 + i] = 10 * s_q[i, c*128 + p], q = 0:v, 1:l
    sT10c = nc.declare_dram_parameter("sT10c", [P, 2 * B], f32, isOutput=False)
    # wint[p, c*256 + 2i] = pos[i, c*128+p]; [.., 2i+1] = neg[i, c*128+p]
    wint = nc.declare_dram_parameter("wint", [P, 2 * B], bf16, isOutput=False)
    posf = nc.declare_dram_parameter("posf", [ROWS, B], f32, isOutput=False)
    wv = nc.declare_dram_parameter("wv", [ROWS, B], f32, isOutput=False)
    wl = nc.declare_dram_parameter("wl", [ROWS, B], f32, isOutput=False)
    # grid constants
    grid10 = nc.declare_dram_parameter("grid10", [P, G], f32, isOutput=False)
    tdT = nc.declare_dram_parameter("tdT", [G, G], f32, isOutput=False)
    bias2 = nc.declare_dram_parameter("bias2", [G, 1], f32, isOutput=False)
    out = nc.declare_dram_parameter("out", [ROWS, B], f32, isOutput=True)

    with TileContext(nc) as tc:
        with (
            tc.tile_pool(name="const", bufs=1) as cpool,
            tc.tile_pool(name="bcast", bufs=3) as bpool,
            tc.tile_pool(name="bc2", bufs=4) as bcpool,
            tc.tile_pool(name="diff", bufs=3) as diffpool,
            tc.tile_pool(name="sig", bufs=3) as sigpool,
            tc.tile_pool(name="phi", bufs=7) as phipool,
            tc.tile_pool(name="fsb", bufs=3) as fbpool,
            tc.tile_pool(name="fin", bufs=1) as fpool,
            tc.tile_pool(name="psum_f", bufs=2, space="PSUM") as ppool_f,
            tc.tile_pool(name="psum_a", bufs=1, space="PSUM") as ppool_a,
        ):
            # --- load resident inputs ---
            sT_t = cpool.tile([P, 2 * B], f32, tag="sT")
            wint_t = cpool.tile([P, 2 * B], bf16, tag="wint")
            grid10_t = cpool.tile([P, G], f32, tag="grid10")
            tdT_t = cpool.tile([G, G], f32, tag="tdT")
            bias2_t = cpool.tile([G, 1], f32, tag="bias2")
            nc.sync.dma_start(out=sT_t[:], in_=sT10c[:])
            nc.sync.dma_start(out=wint_t[:], in_=wint[:])
            nc.sync.dma_start(out=grid10_t[:], in_=grid10[:])
            nc.sync.dma_start(out=tdT_t[:], in_=tdT[:])
            nc.sync.dma_start(out=bias2_t[:], in_=bias2[:])

            # Sp/Sn destination tiles (filled row by row)
            sp_v = fpool.tile([P, B], f32, tag="sp_v")
            sn_v = fpool.tile([P, B], f32, tag="sn_v")
            sp_l = fpool.tile([P, B], f32, tag="sp_l")
            sn_l = fpool.tile([P, B], f32, tag="sn_l")

            # Software-pipelined row loop: every engine is in-order, so a
            # naive per-row emission serializes the whole dependency chain
            # (DVE idles between diff and the evict copy of the same row).
            # Each stage runs at a fixed row lag; at any iteration every
            # emitted instruction's producers were emitted >= 1 iteration
            # earlier, so no engine queue ever blocks in steady state.
            # Lags: bcast 0 | diff 1 | sg2/phi 2 | Fmm 3 | Fsb 4 | Td 5 |
            #       F2sb 6 | recon 7 | stg 8 | scatter 9
            tl = {k: {} for k in ("bc2", "diff", "sg2", "phi", "Fps", "Fsb",
                                  "Fps2", "F2sb", "acc2", "stg")}
            for it in range(ROWS + 9):
                r = it
                if r < ROWS:  # S0: stage + broadcast
                    stage = bpool.tile([1, 2 * B], f32, tag="stage")
                    nc.sync.dma_start(out=stage[:, 0:B], in_=sv[r : r + 1, :])
                    nc.sync.dma_start(out=stage[:, B : 2 * B], in_=sl[r : r + 1, :])
                    bc2 = bcpool.tile([P, 2 * B], f32, tag="bc")
                    nc.gpsimd.partition_broadcast(bc2[:], stage[:])
                    tl["bc2"][r] = bc2
                x = it - 8
                if 0 <= x < ROWS:  # S9: evict (first in DVE queue)
                    stg = bpool.tile([2, 2 * B], f32, tag="stg")
                    nc.vector.tensor_copy(stg[:], tl["acc2"].pop(x)[:])
                    tl["stg"][x] = stg
                x = it - 9
                if 0 <= x < ROWS:  # S10: row scatter
                    stg = tl["stg"].pop(x)
                    nc.sync.dma_start(out=sp_v[x : x + 1, :], in_=stg[0:1, 0:B])
                    nc.sync.dma_start(out=sn_v[x : x + 1, :], in_=stg[1:2, 0:B])
                    nc.sync.dma_start(out=sp_l[x : x + 1, :], in_=stg[0:1, B : 2 * B])
                    nc.sync.dma_start(out=sn_l[x : x + 1, :], in_=stg[1:2, B : 2 * B])
                x = it - 1
                if 0 <= x < ROWS:  # S1: (k, m) outer difference, one DVE op
                    diff_t = diffpool.tile([P, 2 * B], f32, tag="diff")
                    scols = sT_t[:, x : 2 * B : P].broadcast_to((P, 2 * NCHUNK, G))
                    gbc = grid10_t[:, None, :].broadcast_to((P, 2 * NCHUNK, G))
                    nc.vector.tensor_sub(
                        diff_t[:].rearrange("p (c m) -> p c m", c=2 * NCHUNK),
                        scols,
                        gbc,
                    )
                    tl["diff"][x] = diff_t
                x = it - 2
                if 0 <= x < ROWS:  # S2: pass-1 sigmoid; S7: pass-2 kernel
                    sg2 = sigpool.tile([P, 2 * B], bf16, tag="sg2")
                    nc.scalar.activation(
                        out=sg2[:],
                        in_=tl["diff"].pop(x)[:],
                        func=mybir.ActivationFunctionType.Sigmoid,
                        bias=0.0,
                        scale=1.0,
                    )
                    tl["sg2"][x] = sg2
                    phi = phipool.tile([G, 2 * B], bf16, tag="phi")
                    nc.scalar.activation(
                        out=phi[:],
                        in_=tl["bc2"].pop(x)[:],
                        func=mybir.ActivationFunctionType.Sigmoid,
                        bias=bias2_t[:, 0:1],
                        scale=-1.0 / W,
                    )
                    tl["phi"][x] = phi
                x = it - 3
                if 0 <= x < ROWS:  # S3: grid-F matmuls
                    sg2 = tl["sg2"][x]
                    Fps = ppool_f.tile([G, 4], f32, tag="Fps")
                    for q in (0, 2):
                        for c in range(NCHUNK):
                            nc.tensor.matmul(
                                out=Fps[:, q : q + 2],
                                lhsT=sg2[:, (q * 4 + c) * G : (q * 4 + c + 1) * G],
                                rhs=wint_t[:, c * 256 + 2 * x : c * 256 + 2 * x + 2],
                                start=(c == 0),
                                stop=(c == NCHUNK - 1),
                            )
                    tl["Fps"][x] = Fps
                x = it - 4
                if 0 <= x < ROWS:  # S4: F to SBUF
                    tl["sg2"].pop(x)
                    Fsb = fbpool.tile([G, 4], f32, tag="Fsb")
                    nc.vector.tensor_copy(Fsb[:], tl["Fps"].pop(x)[:])
                    tl["Fsb"][x] = Fsb
                x = it - 5
                if 0 <= x < ROWS:  # S5: banded difference+sharpening filter
                    Fps2 = ppool_f.tile([G, 4], f32, tag="Fps2")
                    nc.tensor.matmul(
                        out=Fps2[:], lhsT=tdT_t[:], rhs=tl["Fsb"].pop(x)[:],
                        start=True, stop=True,
                    )
                    tl["Fps2"][x] = Fps2
                x = it - 6
                if 0 <= x < ROWS:  # S6: filtered F to SBUF as bf16
                    F2sb = fbpool.tile([G, 4], bf16, tag="F2sb")
                    nc.vector.tensor_copy(F2sb[:], tl["Fps2"].pop(x)[:])
                    tl["F2sb"][x] = F2sb
                x = it - 7
                if 0 <= x < ROWS:  # S8: reconstruction matmuls
                    F2sb = tl["F2sb"].pop(x)
                    phi = tl["phi"].pop(x)
                    acc2 = ppool_a.tile([2, 2 * B], f32, tag="acc")
                    for (q, moff) in ((0, 0), (2, B)):
                        for h2 in (0, 512):
                            nc.tensor.matmul(
                                out=acc2[:, moff + h2 : moff + h2 + 512],
                                lhsT=F2sb[:, q : q + 2],
                                rhs=phi[:, moff + h2 : moff + h2 + 512],
                                start=True,
                                stop=True,
                            )
                    tl["acc2"][x] = acc2

            # --- finals, batched over all 128 rows ---
            pos_t = fpool.tile([ROWS, B], f32, tag="pos")
            wv_t = fpool.tile([ROWS, B], f32, tag="wv")
            wl_t = fpool.tile([ROWS, B], f32, tag="wl")
            nc.sync.dma_start(out=pos_t[:], in_=posf[:])
            nc.sync.dma_start(out=wv_t[:], in_=wv[:])
            nc.sync.dma_start(out=wl_t[:], in_=wl[:])

            res = fpool.tile([ROWS, B], f32, tag="res")
            for (sp, sn, w_t, dst) in (
                (sp_v, sn_v, wv_t, None),
                (sp_l, sn_l, wl_t, res),
            ):
                d1 = fpool.tile([ROWS, B], f32, tag="d1")
                nc.vector.tensor_sub(d1[:], sn[:], sp[:])
                nc.vector.tensor_mul(d1[:], pos_t[:], d1[:])
                nc.vector.tensor_add(d1[:], d1[:], sp[:])
                # den = K + 1 + rank_minus_1 = 61 + d1
                nc.vector.tensor_scalar_add(d1[:], d1[:], K + 1.0)
                nc.vector.reciprocal(d1[:], d1[:])
                if dst is None:
                    t_v = fpool.tile([ROWS, B], f32, tag="t_v")
                    nc.vector.tensor_mul(t_v[:], w_t[:], d1[:])
                else:
                    nc.vector.tensor_mul(d1[:], w_t[:], d1[:])
                    nc.vector.tensor_add(res[:], t_v[:], d1[:])
            nc.vector.tensor_scalar_mul(res[:], res[:], K + 1.0)
            nc.sync.dma_start(out=out[:], in_=res[:])

    nc.compile()
    return nc


_NC_CACHE = None


def _get_nc():
    global _NC_CACHE
    if _NC_CACHE is None:
        _NC_CACHE = _build_bass()
    return _NC_CACHE


def _prep_core_inputs(s_v, s_l, pos_f, neg_f, w_v, w_l, core):
    import ml_dtypes

    lo, hi = core * ROWS, (core + 1) * ROWS
    svs = np.ascontiguousarray(s_v[lo:hi])
    sls = np.ascontiguousarray(s_l[lo:hi])

    def t10(x):
        # [p, c*128 + i] = 10 * x[i, c*128 + p]
        y = x.reshape(ROWS, NCHUNK, P)  # [i, c, p]
        return (10.0 * y).transpose(2, 1, 0).reshape(P, B)

    ps = pos_f[lo:hi]
    ns = neg_f[lo:hi]
    # wint[p, c*256 + 2i] = pos[i, c*128+p]; odd = neg
    wint = np.empty((P, NCHUNK, 2 * P), np.float32)
    pT = ps.reshape(ROWS, NCHUNK, P).transpose(2, 1, 0)  # [p, c, i]
    nT = ns.reshape(ROWS, NCHUNK, P).transpose(2, 1, 0)
    wint[:, :, 0::2] = pT
    wint[:, :, 1::2] = nT
    return {
        "sv": svs.astype(np.float32),
        "sl": sls.astype(np.float32),
        "sT10c": np.ascontiguousarray(
            np.concatenate([t10(svs), t10(sls)], axis=1).astype(np.float32)
        ),
        "wint": wint.reshape(P, 2 * B).astype(ml_dtypes.bfloat16),
        "posf": np.ascontiguousarray(ps),
        "wv": np.ascontiguousarray(w_v[lo:hi]).astype(np.float32),
        "wl": np.ascontiguousarray(w_l[lo:hi]).astype(np.float32),
        "grid10": GRID10,
        "tdT": TDT,
        "bias2": BIAS2,
    }


def _run(in_maps, trace=False):
    from concourse.bass_utils import run_bass_kernel_spmd

    nc = _get_nc()
    return run_bass_kernel_spmd(nc, in_maps, core_ids=list(range(NCORES)), trace=trace)


def kernel(s_v, s_l, pos_mask, neg_mask, w_v, w_l, _trace=False):
    pos_f = pos_mask.astype(np.float32)
    neg_f = neg_mask.astype(np.float32)
    in_maps = [
        _prep_core_inputs(s_v, s_l, pos_f, neg_f, w_v, w_l, core)
        for core in range(NCORES)
    ]
    res = _run(in_maps, trace=_trace)
    outs = [res.results[i]["out"] for i in range(NCORES)]
    full = np.concatenate(outs, axis=0).astype(np.float32)
    if _trace:
        return full, res
    return full
